# revision 1
# baseline (speedup 1.0000x reference)
"""Trainium2 Bass kernel for nn_DiffeqExactTraceAttention.

Strategy: data-parallel over batch B=8 across the 8 NeuronCores (one batch
element per core, attention over N=256 fully local, weights replicated).

Per-core computation (all activations stored transposed, [feat, token]):
  query MADE-MLP -> qT [2048, 256]; k/v tanh-MLPs -> kT, vT [128, 256]
  per dim d (16): scoresT[m,n] per head via PE, exp on ACT (no max needed:
  |scores| < 1), diagonal zeroed via affine_select, o + softmax denominator
  from one matmul against [v' | 1], per-partition normalize, PE transpose,
  projection + dimwise 4-layer MLP forward and JVP (diagonal Jacobian).

Outputs y, jac [B, N, D] (d_b3 added host-side).
"""

import os
import sys
import threading

import numpy as np

sys.path.insert(0, "/opt/trn_rl_repo")

import concourse.bass as bass  # noqa: E402
import concourse.mybir as mybir  # noqa: E402
import concourse.tile as tile  # noqa: E402
from concourse import bacc  # noqa: E402

F32 = mybir.dt.float32
F32R = mybir.dt.float32r
AF = mybir.ActivationFunctionType
ALU = mybir.AluOpType

B, N, D = 8, 256, 16
HID, H, DH, NH = 256, 128, 64, 4
dh = H // NH  # 32

_lock = threading.Lock()
_cache = {}


def _emit(nc, tc, ins, out):
    """Emit the per-core kernel. `ins` maps name -> dram AP."""
    from contextlib import ExitStack

    with ExitStack() as ctx:
        cw = ctx.enter_context(tc.tile_pool(name="cw", bufs=1))
        sb1 = ctx.enter_context(tc.tile_pool(name="sb1", bufs=1))
        work = ctx.enter_context(tc.tile_pool(name="work", bufs=3))
        epool = ctx.enter_context(tc.tile_pool(name="epool", bufs=3))
        psum = ctx.enter_context(tc.tile_pool(name="psum", bufs=1, space="PSUM"))

        # ---- HAM warm-up: an uninterrupted fp32 matmul burst (>3.4us)
        # trips the PE clock gate to 2.4 GHz while the weight DMAs stream;
        # the dense body keeps it warm (no idle window exceeds ~3.4us).
        wdat = sb1.tile([128, 512], F32, tag="wdat")
        nc.vector.memset(wdat, 0.001)
        pwu = psum.tile([128, 512], F32, tag="ps", bufs=4, name="pwu")
        for _ in range(12):
            nc.tensor.matmul(pwu, wdat[:, 0:128], wdat, start=True, stop=True)

        # ---- load constants/weights to SBUF (critical-first, big tensors
        # split into parallel chunk DMAs) ----
        def load(name, split=False):
            ap = ins[name]
            t = cw.tile(list(ap.shape), ap.dtype, tag=name)
            if split and len(ap.shape) == 3:
                for kc in range(ap.shape[1]):
                    m = ap.shape[2]
                    if m >= 2048:
                        for q4 in range(4):
                            nc.sync.dma_start(
                                out=t[:, kc, bass.ts(q4, m // 4)],
                                in_=ap[:, kc, bass.ts(q4, m // 4)])
                    elif m >= 1024:
                        nc.sync.dma_start(out=t[:, kc, 0:m // 2],
                                          in_=ap[:, kc, 0:m // 2])
                        nc.sync.dma_start(out=t[:, kc, m // 2:],
                                          in_=ap[:, kc, m // 2:])
                    else:
                        nc.sync.dma_start(out=t[:, kc], in_=ap[:, kc])
            elif split and len(ap.shape) == 2:
                half = ap.shape[1] // 2
                nc.sync.dma_start(out=t[:, 0:half], in_=ap[:, 0:half])
                nc.sync.dma_start(out=t[:, half:], in_=ap[:, half:])
            else:
                nc.sync.dma_start(out=t, in_=ap)
            return t

        order1 = ["xT", "w0q", "qb0", "w1q", "qb1", "w2q", "kw0", "kb0",
                  "vw0", "vb0", "hmask", "ones1", "ident", "blockones",
                  "xflat"]
        order2 = ["kw1", "kb1", "vw1", "vb1"]
        order3 = ["qb2", "kw2", "kb2", "vw2", "vb2", "pw", "pb"]
        order4 = ["w1z", "b0p", "w0xpm", "dw1", "db1", "dw2", "db2", "w3ab"]
        t = {}
        for n in order1:
            t[n] = load(n, split=n in ("w2q", "w1q"))
        for n in order2 + order3 + order4:
            t[n] = load(n, split=n in ("w1q", "w2q", "kw1", "vw1", "dw1",
                                       "dw2"))
        xT = t["xT"]

        # ---- stage 1: query / key / value nets ----
        hq1 = sb1.tile([128, 2, 256], F32R, tag="hq1")
        hq2 = sb1.tile([128, 2, 256], mybir.dt.float16, tag="hq2")
        qT = sb1.tile([128, 16, 256], F32R, tag="qT")
        kT = sb1.tile([128, 256], F32R, tag="kT")
        vT = sb1.tile([128, 256], F32, tag="vT")

        for mc in range(2):
            p = psum.tile([128, 256], F32, tag="ps", bufs=4, name="p1")
            nc.tensor.matmul(p, t["w0q"][:, bass.ts(mc, 128)], xT,
                             start=True, stop=True)
            nc.vector.tensor_scalar(out=hq1[:, mc], in0=p,
                                    scalar1=t["qb0"][:, mc:mc + 1],
                                    scalar2=0.0, op0=ALU.add, op1=ALU.max)
        for mc in range(2):
            p = psum.tile([128, 256], F32, tag="ps", bufs=4, name="p1")
            for kc in range(2):
                nc.tensor.matmul(p, t["w1q"][:, kc, bass.ts(mc, 128)],
                                 hq1[:, kc], start=(kc == 0), stop=(kc == 1))
            nc.vector.tensor_scalar(out=hq2[:, mc], in0=p,
                                    scalar1=t["qb1"][:, mc:mc + 1],
                                    scalar2=0.0, op0=ALU.add, op1=ALU.max)
        for g in range(4):
            p = psum.tile([128, 1024], F32, tag="pscore", bufs=2, name="pq")
            for sub in range(4):
                mc = g * 4 + sub
                for kc in range(2):
                    nc.tensor.matmul(p[:, bass.ts(sub, 256)],
                                     t["w2q"][:, kc, bass.ts(mc, 128)],
                                     hq2[:, kc], start=(kc == 0),
                                     stop=(kc == 1))
            qb2v = bass.AP(tensor=t["qb2"].tensor,
                           offset=t["qb2"].offset + g * 4,
                           ap=[[16, 128], [1, 4], [0, 256]])
            nc.vector.tensor_add(qT[:, g * 4:(g + 1) * 4, :],
                                 p.rearrange("p (s n) -> p s n", n=256), qb2v)

        for (w0, w1, w2, b0, b1, b2, outT) in (
            ("kw0", "kw1", "kw2", "kb0", "kb1", "kb2", kT),
            ("vw0", "vw1", "vw2", "vb0", "vb1", "vb2", vT),
        ):
            h1 = work.tile([128, 2, 256], F32R, tag="kv1")
            h2 = work.tile([128, 2, 256], F32R, tag="kv2")
            p = psum.tile([128, 1024], F32, tag="pscore", bufs=2, name="pkv")
            for mc in range(2):
                nc.tensor.matmul(p[:, bass.ts(mc, 256)],
                                 t[w0][:, bass.ts(mc, 128)], xT,
                                 start=True, stop=True)
            for mc in range(2):
                nc.scalar.activation(h1[:, mc], p[:, bass.ts(mc, 256)],
                                     AF.Tanh, bias=t[b0][:, mc:mc + 1])
            p2 = psum.tile([128, 1024], F32, tag="pscore", bufs=2, name="pkv2")
            for mc in range(2):
                for kc in range(2):
                    nc.tensor.matmul(p2[:, bass.ts(mc, 256)],
                                     t[w1][:, kc, bass.ts(mc, 128)],
                                     h1[:, kc], start=(kc == 0),
                                     stop=(kc == 1))
            for mc in range(2):
                nc.scalar.activation(h2[:, mc], p2[:, bass.ts(mc, 256)],
                                     AF.Tanh, bias=t[b1][:, mc:mc + 1])
            p = psum.tile([128, 256], F32, tag="ps", bufs=4, name="p1")
            for kc in range(2):
                nc.tensor.matmul(p, t[w2][:, kc, :], h2[:, kc],
                                 start=(kc == 0), stop=(kc == 1))
            nc.vector.tensor_scalar_add(out=outT, in0=p, scalar1=t[b2][:, 0:1])

        # Per-head masked copies of kT (other heads' rows zeroed) so the
        # scores matmuls contract over the full 128 partitions at base 0
        # (hardware rejects tile_position/base-partition offsets here).
        kTm = []
        for hh in range(4):
            km = sb1.tile([128, 256], F32R, tag=f"kTm{hh}")
            nc.vector.tensor_scalar_mul(out=km, in0=kT,
                                        scalar1=t["hmask"][:, hh:hh + 1])
            kTm.append(km)

        # Diagonal scores for ALL d upfront: prodAll[.,d,.] = qT[.,d,.]*kT,
        # snn[n, 4h] per n-chunk via block-ones matmul, one exp per chunk.
        prodAll = sb1.tile([128, 16, 256], F32, tag="prodAll")
        kTb = bass.AP(tensor=kT.tensor, offset=kT.offset,
                      ap=[[256, 128], [0, 16], [1, 256]])
        nc.vector.tensor_mul(prodAll, qT, kTb)
        ed_all = []
        for ns in range(2):
            psn = psum.tile([128, 64], F32, tag="ps", bufs=4, name="psn")
            for d in range(D):
                nc.tensor.matmul(psn[:, d * 4:(d + 1) * 4],
                                 prodAll[:, d, bass.ts(ns, 128)],
                                 t["blockones"], start=True, stop=True)
            eda = sb1.tile([128, 64], F32, tag=f"ed_{ns}")
            nc.scalar.activation(eda, psn, AF.Exp)
            ed_all.append(eda)

        # v' = vT transposed, augmented with ones cols: v1_mj [128, 4*33]
        v1s = []
        for mj in range(2):
            pt = psum.tile([128, 128], F32, tag="ps", bufs=4, name="ptv")
            nc.tensor.transpose(pt, vT[:, bass.ts(mj, 128)], t["ident"])
            v1 = sb1.tile([128, 132], F32, tag=f"v1_{mj}")
            for hh in range(4):
                nc.vector.tensor_copy(out=v1[:, hh * 33:hh * 33 + 32],
                                      in_=pt[:, bass.ts(hh, 32)])
            ones_view = v1.rearrange("p (h t) -> p h t", t=33)[:, :, 32:33]
            nc.vector.tensor_copy(out=ones_view,
                                  in_=t["ones1"].to_broadcast([128, 4, 1]))
            v1s.append(v1)
        v1bs = []
        for mj in range(2):
            v1b = sb1.tile([128, 132], mybir.dt.float16, tag=f"v1b_{mj}")
            nc.vector.tensor_copy(out=v1b, in_=v1s[mj])
            v1bs.append(v1b)

        # ---- stage 2: software-pipelined attention + dimwise ----
        # Every PE consumer lags >=1 step behind its producer chain so the
        # in-order PE stream never waits on ACT/DVE latency: per step d we
        # emit scores(d), mm_o(d-1), transposes/hfeat(d-3), and staggered
        # dimwise layer stages. A dense PE stream keeps the HAM clock warm.
        yj = sb1.tile([2, D * N], F32, tag="yj")
        es_st = {}
        oA_st = {}
        oTd2_st = {}
        z0_st = {}
        ag1_st = {}
        ag2_st = {}
        ag3_st = {}

        def SC2(p_):
            es = {}
            for mj in range(2):
                for half in range(2):
                    ps = psum.tile([128, 1024], F32, tag="pscore", bufs=2,
                                   name="pscore")
                    for hi in range(2):
                        hh = half * 2 + hi
                        nc.tensor.matmul(
                            ps[:, bass.ts(hi, 512)],
                            kTm[hh][:, bass.ts(mj, 128)],
                            qT[:, 2 * p_:2 * p_ + 2, :],
                            start=True, stop=True)
                    e = epool.tile([128, 1024], mybir.dt.float16, tag="e",
                                   bufs=9, name="e")
                    nc.scalar.activation(e, ps, AF.Exp)
                    es[(mj, half)] = e
            es_st[p_] = es

        def MO(d):
            p_, di = d // 2, d % 2
            es = es_st[p_]
            poA = psum.tile([128, 264], F32, tag="ps", bufs=4, name="poA")
            for ns in range(2):
                for hh in range(4):
                    for mj in range(2):
                        e = es[(mj, hh // 2)]
                        o0 = (hh % 2) * 512 + di * 256 + ns * 128
                        nc.tensor.matmul(
                            poA[:, ns * 132 + hh * 33:ns * 132 + (hh + 1) * 33],
                            e[:, o0:o0 + 128],
                            v1bs[mj][:, hh * 33:(hh + 1) * 33],
                            start=(mj == 0), stop=(mj == 1))
            if di == 1:
                del es_st[p_]
            # batched diag-correction + normalize over both n-chunks
            oAc = work.tile([128, 264], F32, tag="oAc", bufs=4, name="oAc")
            edv = bass.AP(tensor=ed_all[0].tensor,
                          offset=ed_all[0].offset + d * 4,
                          ap=[[64, 128], [0, 2], [1, 4], [0, 33]])
            edv2 = bass.AP(tensor=ed_all[1].tensor,
                           offset=ed_all[1].offset + d * 4,
                           ap=[[64, 128], [1, 4], [0, 33]])
            oAcv = oAc.rearrange("p (ns h t) -> p ns h t", ns=2, t=33)
            nc.vector.tensor_mul(
                oAcv[:, 0:1], v1s[0].rearrange("p (h t) -> p h t", t=33).unsqueeze(1),
                bass.AP(tensor=ed_all[0].tensor,
                        offset=ed_all[0].offset + d * 4,
                        ap=[[64, 128], [1, 1], [1, 4], [0, 33]]))
            nc.vector.tensor_mul(
                oAcv[:, 1:2], v1s[1].rearrange("p (h t) -> p h t", t=33).unsqueeze(1),
                bass.AP(tensor=ed_all[1].tensor,
                        offset=ed_all[1].offset + d * 4,
                        ap=[[64, 128], [1, 1], [1, 4], [0, 33]]))
            nc.vector.tensor_sub(oAc, oAc, poA)
            rinv = work.tile([128, 8], F32, tag="rinv", bufs=4, name="rinv")
            nc.vector.reciprocal(rinv.rearrange("p (ns h) -> p ns h", ns=2),
                                 oAcv[:, :, :, 32:33])
            oA = work.tile([128, 256], F32, tag="oA", bufs=4, name="oA")
            nc.vector.tensor_mul(
                oA.rearrange("p (ns h c) -> p ns h c", ns=2, c=32),
                oAcv[:, :, :, 0:32],
                rinv.rearrange("p (ns h) -> p ns h", ns=2).to_broadcast(
                    [128, 2, 4, 32]))
            oA_st[d] = oA

        def T(d):
            p_, di = d // 2, d % 2
            if di == 0:
                oTd2_st[p_] = work.tile([128, 512], F32R, tag="oTd2", bufs=3,
                                        name="oTd2")
            oTd2 = oTd2_st[p_]
            pt = psum.tile([128, 256], F32, tag="ps", bufs=4, name="pt")
            for ns in range(2):
                nc.tensor.transpose(pt[:, bass.ts(ns, 128)],
                                    oA_st[d][:, bass.ts(ns, 128)],
                                    t["ident"])
            nc.vector.tensor_copy(
                out=oTd2[:, di * 256:(di + 1) * 256], in_=pt)
            del oA_st[d]
            if di == 1:
                ph = psum.tile([64, 512], F32, tag="ps", bufs=4, name="ph")
                nc.tensor.matmul(ph, t["pw"], oTd2, start=True, stop=True)
                z0 = work.tile([65, 512], F32R, tag="z0", bufs=3, name="z0")
                nc.vector.tensor_scalar_add(out=z0[0:64, :], in0=ph,
                                            scalar1=t["pb"][:, 0:1])
                nc.gpsimd.tensor_copy(out=z0[64:65, :],
                                      in_=t["xflat"][:, bass.ts(p_, 512)])
                z0_st[p_] = z0
                del oTd2_st[p_]

        def L1(p_):
            z0 = z0_st.pop(p_)
            a1 = work.tile([128, 2, 512], F32R, tag="a1", bufs=2, name="a1")
            g1 = work.tile([128, 2, 512], F32R, tag="g1", bufs=2, name="g1")
            for mc in range(2):
                pdm = psum.tile([128, 512], F32, tag="ps", bufs=4, name="pdm")
                nc.tensor.matmul(pdm, t["w1z"][:, bass.ts(mc, 128)], z0,
                                 start=True, stop=True)
                nc.scalar.activation(a1[:, mc], pdm, AF.Tanh,
                                     bias=t["b0p"][:, mc:mc + 1])
            for mc in range(2):
                nc.gpsimd.tensor_mul(g1[:, mc], a1[:, mc], a1[:, mc])
            for mc in range(2):
                nc.gpsimd.tensor_scalar(
                    out=g1[:, mc], in0=g1[:, mc],
                    scalar1=t["w0xpm"][:, mc:mc + 1],
                    scalar2=t["w0xpm"][:, 2 + mc:3 + mc],
                    op0=ALU.mult, op1=ALU.add)
            ag1_st[p_] = (a1, g1)

        def L2(p_):
            a1, g1 = ag1_st.pop(p_)
            a2 = work.tile([128, 2, 512], F32R, tag="a2", bufs=2, name="a2")
            g2 = work.tile([128, 2, 512], F32R, tag="g2", bufs=2, name="g2")
            pgs = []
            for mc in range(2):
                pa = psum.tile([128, 512], F32, tag="ps", bufs=4, name="pdm")
                pg = psum.tile([128, 512], F32, tag="ps", bufs=4, name="pdg")
                for kc in range(2):
                    nc.tensor.matmul(pa, t["dw1"][:, kc, bass.ts(mc, 128)],
                                     a1[:, kc], start=(kc == 0),
                                     stop=(kc == 1))
                for kc in range(2):
                    nc.tensor.matmul(pg, t["dw1"][:, kc, bass.ts(mc, 128)],
                                     g1[:, kc], start=(kc == 0),
                                     stop=(kc == 1))
                nc.scalar.activation(a2[:, mc], pa, AF.Tanh,
                                     bias=t["db1"][:, mc:mc + 1])
                pgs.append(pg)
            for mc in range(2):
                nc.gpsimd.tensor_mul(g2[:, mc], a2[:, mc], a2[:, mc])
            for mc in range(2):
                nc.vector.scalar_tensor_tensor(
                    out=g2[:, mc], in0=g2[:, mc], scalar=1.0, in1=pgs[mc],
                    op0=ALU.subtract, op1=ALU.mult)
            ag2_st[p_] = (a2, g2)

        def L3(p_):
            a2, g2 = ag2_st.pop(p_)
            a3 = work.tile([128, 512], F32R, tag="a3", bufs=2, name="a3")
            g3 = work.tile([128, 512], F32R, tag="g3", bufs=2, name="g3")
            pa = psum.tile([128, 512], F32, tag="ps", bufs=4, name="pdm")
            pg = psum.tile([128, 512], F32, tag="ps", bufs=4, name="pdg")
            for kc in range(2):
                nc.tensor.matmul(pa, t["dw2"][:, kc, :], a2[:, kc],
                                 start=(kc == 0), stop=(kc == 1))
            for kc in range(2):
                nc.tensor.matmul(pg, t["dw2"][:, kc, :], g2[:, kc],
                                 start=(kc == 0), stop=(kc == 1))
            nc.scalar.activation(a3, pa, AF.Tanh, bias=t["db2"][:, 0:1])
            nc.gpsimd.tensor_mul(g3, a3, a3)
            nc.vector.scalar_tensor_tensor(
                out=g3, in0=g3, scalar=1.0, in1=pg,
                op0=ALU.subtract, op1=ALU.mult)
            ag3_st[p_] = (a3, g3)

        def L4(p_):
            a3, g3 = ag3_st.pop(p_)
            py = psum.tile([2, 512], F32, tag="ps", bufs=4, name="py")
            nc.tensor.matmul(py, t["w3ab"][:, 0:2], a3, start=True,
                             stop=False)
            nc.tensor.matmul(py, t["w3ab"][:, 2:4], g3, start=False,
                             stop=True)
            nc.vector.tensor_copy(out=yj[:, bass.ts(p_, 512)], in_=py)

        # schedule: SC(d); MO(d-1); T(d-3); L1(p)@d=2p+5, L2@2p+6,
        # L3@2p+7, L4@2p+8
        def dw_stages(d):
            for stage, off in ((L1, 4), (L2, 5), (L3, 6), (L4, 7)):
                if d >= off and (d - off) % 2 == 0:
                    p_ = (d - off) // 2
                    if p_ < D // 2:
                        stage(p_)

        for d in range(0, D + 9):
            if d % 2 == 0 and d // 2 < D // 2:
                SC2(d // 2)
            if 0 <= d - 1 < D:
                MO(d - 1)
            if 0 <= d - 2 < D:
                T(d - 2)
            dw_stages(d)
        nc.sync.dma_start(out=out, in_=yj)


def _build():
    nc = bacc.Bacc("TRN2", target_bir_lowering=False, debug=False)
    shapes = {
        "xT": [D, N], "xflat": [1, D * N], "w0q": [D, HID], "w1q": [128, 2, HID],
        "w2q": [128, 2, D * H], "kw0": [D, HID], "kw1": [128, 2, HID],
        "kw2": [128, 2, H], "vw0": [D, HID], "vw1": [128, 2, HID],
        "vw2": [128, 2, H], "pw": [H, DH], "w1z": [65, HID],
        "dw1": [128, 2, HID], "dw2": [128, 2, H], "w3ab": [H, 4],
        "qb0": [128, 2], "qb1": [128, 2], "qb2": [128, 16],
        "kb0": [128, 2], "kb1": [128, 2], "kb2": [128, 1],
        "vb0": [128, 2], "vb1": [128, 2], "vb2": [128, 1],
        "pb": [DH, 1], "b0p": [128, 2], "w0xpm": [128, 4],
        "db1": [128, 2], "db2": [128, 1], "ident": [128, 128],
        "blockones": [128, 4], "hmask": [128, 4], "ones1": [128, 1],
    }
    f32r_names = {"xT", "xflat", "w0q", "w1q", "kw0", "kw1", "kw2",
                  "vw0", "vw1", "vw2", "pw", "w1z", "dw1", "dw2", "w3ab"}

    def _dt(n):
        if n == "w2q":
            return mybir.dt.float16
        return F32R if n in f32r_names else F32
    ins = {n: nc.dram_tensor(n, s, _dt(n), kind="ExternalInput").ap()
           for n, s in shapes.items()}
    out = nc.dram_tensor("out", [2, D * N], F32, kind="ExternalOutput").ap()
    with tile.TileContext(nc) as tc:
        _emit(nc, tc, ins, out)
    nc.finalize()
    return nc


def _col2(v):
    # [256] -> [128, 2] with column mc = chunk mc
    return np.ascontiguousarray(v.reshape(2, 128).T)


def _prep_inputs(t, x, q_w0, q_b0, k_w0, k_b0, v_w0, v_b0, q_w1, q_b1, k_w1,
                 k_b1, v_w1, v_b1, q_w2, q_b2, k_w2, k_b2, v_w2, v_b2, p_w,
                 p_b, d_w0, d_b0, d_w1, d_b1, d_w2, d_b2, d_w3, d_b3, q_m0,
                 q_m1, q_m2):
    f = np.float32
    scale = f(1.0 / np.sqrt(dh))

    def kchunk(w):  # [256, M] -> [128, 2, M]
        return np.ascontiguousarray(w.reshape(2, 128, -1).transpose(1, 0, 2))

    W0q = (q_w0 * q_m0).astype(f)
    W1q = (q_w1 * q_m1).astype(f)
    W2q = (q_w2 * q_m2).astype(f)
    b0p = (d_b0 + t[0] * d_w0[0]).astype(f)
    w0x = d_w0[1].astype(f)
    W1z = np.concatenate([d_w0[2:], d_w0[1:2]], axis=0).astype(f)
    w3ab = np.zeros((H, 4), f)
    w3ab[:, 0] = d_w3[:, 0]
    w3ab[:, 3] = d_w3[:, 0]
    w0xpm = np.concatenate([_col2(-w0x), _col2(w0x)], axis=1)

    shared = {
        "w0q": W0q, "w1q": kchunk(W1q),
        "w2q": kchunk(W2q).astype(np.float16),
        "kw0": k_w0.astype(f), "kw1": kchunk(k_w1.astype(f)),
        "kw2": kchunk((k_w2 * scale).astype(f)),
        "vw0": v_w0.astype(f), "vw1": kchunk(v_w1.astype(f)),
        "vw2": kchunk(v_w2.astype(f)),
        "pw": p_w.astype(f), "w1z": W1z, "dw1": kchunk(d_w1.astype(f)),
        "dw2": kchunk(d_w2.astype(f)), "w3ab": w3ab,
        "qb0": _col2(q_b0.astype(f)), "qb1": _col2(q_b1.astype(f)),
        "qb2": np.ascontiguousarray(q_b2.astype(f).reshape(16, 128).T),
        "kb0": _col2(k_b0.astype(f)), "kb1": _col2(k_b1.astype(f)),
        "kb2": (k_b2 * scale).astype(f).reshape(128, 1),
        "vb0": _col2(v_b0.astype(f)), "vb1": _col2(v_b1.astype(f)),
        "vb2": v_b2.astype(f).reshape(128, 1),
        "pb": p_b.astype(f).reshape(DH, 1),
        "b0p": _col2(b0p), "w0xpm": w0xpm,
        "db1": _col2(d_b1.astype(f)), "db2": d_b2.astype(f).reshape(128, 1),
        "ident": np.eye(128, dtype=f),
        "blockones": np.repeat(np.eye(4, dtype=f), 32, axis=0),
        "hmask": np.repeat(np.eye(4, dtype=f), 32, axis=0),
        "ones1": np.ones((128, 1), f),
    }
    shared = {k: np.ascontiguousarray(v) for k, v in shared.items()}
    in_maps = []
    for b in range(B):
        m = dict(shared)
        xt = np.ascontiguousarray(x[b].T.astype(f))
        m["xT"] = xt
        m["xflat"] = xt.reshape(1, -1)
        in_maps.append(m)
    return in_maps, float(d_b3[0])


def kernel(**inputs):
    from concourse.bass_utils import run_bass_kernel_spmd

    inputs = {k: np.asarray(v) for k, v in inputs.items()}
    with _lock:
        if "nc" not in _cache:
            _cache["nc"] = _build()
        nc = _cache["nc"]
    in_maps, b3 = _prep_inputs(**inputs)
    trace = False
    if os.environ.get("KBENCH_TRACE"):
        try:
            import antenv.axon_hooks  # noqa: F401
            trace = True
        except ImportError:
            trace = False
    res = run_bass_kernel_spmd(nc, in_maps, list(range(B)), trace=trace)
    if trace:
        _cache["last_results"] = res
    y = np.zeros((B, N, D), np.float32)
    jac = np.zeros((B, N, D), np.float32)
    for b in range(B):
        o = res.results[b]["out"].reshape(2, D, N)
        y[b] = o[0].T + np.float32(b3)
        jac[b] = o[1].T
    return y, jac



# revision 4
# speedup vs baseline: 1.2509x; 1.2509x over previous
"""Trainium2 Bass kernel for nn_DiffeqExactTraceAttention.

Strategy: data-parallel over batch B=8 across the 8 NeuronCores (one batch
element per core, attention over N=256 fully local, weights replicated).

Per-core computation (all activations stored transposed, [feat, token]):
  query MADE-MLP -> qT [2048, 256]; k/v tanh-MLPs -> kT, vT [128, 256]
  per dim d (16): scoresT[m,n] per head via PE, exp on ACT (no max needed:
  |scores| < 1), diagonal correction via separately-computed diag scores,
  o + softmax denominator from one matmul against [v' | 1], per-partition
  normalize, PE transpose, projection + dimwise 4-layer MLP forward and
  JVP (diagonal Jacobian).

All matmul/DVE traffic is fp16 (PSUM accumulation stays fp32); weights are
packed into two fp16 SBUF blobs + one fp32 bias blob loaded with a handful
of large DMAs so the weight load stays off the critical path.

Outputs y, jac [B, N, D] (d_b3 added host-side).
"""

import os
import sys
import threading

import numpy as np

sys.path.insert(0, "/opt/trn_rl_repo")

import concourse.bass as bass  # noqa: E402
import concourse.mybir as mybir  # noqa: E402
import concourse.tile as tile  # noqa: E402
from concourse import bacc  # noqa: E402

F32 = mybir.dt.float32
F16 = mybir.dt.float16
AF = mybir.ActivationFunctionType
ALU = mybir.AluOpType

B, N, D = 8, 256, 16
HID, H, DH, NH = 256, 128, 64, 4
dh = H // NH  # 32

_lock = threading.Lock()
_cache = {}

# fp16 pack1 segment offsets (cols)
PK1 = {"w1q": 0, "kw1": 512, "vw1": 1024, "kw2": 1536, "vw2": 1792,
       "pw": 2048, "ident": 2112, "blockones": 2240, "ones1": 2244}
PK1_COLS = 2245
# fp16 pack2 segment offsets
PK2 = {"w2q": 0, "dw1": 4096, "dw2": 4608, "w3ab": 4864}
PK2_COLS = 4868
# fp32 bias pack column offsets
PKB = {"qb0": 0, "qb1": 2, "qb2": 4, "kb0": 20, "kb1": 22, "kb2": 24,
       "vb0": 25, "vb1": 27, "vb2": 29, "b0p": 30, "w0xpm": 32,
       "db1": 36, "db2": 38, "hmask": 39}
PKB_COLS = 43


def _emit(nc, tc, ins, out):
    """Emit the per-core kernel. `ins` maps name -> dram AP."""
    from contextlib import ExitStack

    with ExitStack() as ctx:
        cw = ctx.enter_context(tc.tile_pool(name="cw", bufs=1))
        sb1 = ctx.enter_context(tc.tile_pool(name="sb1", bufs=1))
        work = ctx.enter_context(tc.tile_pool(name="work", bufs=3))
        epool = ctx.enter_context(tc.tile_pool(name="epool", bufs=3))
        psum = ctx.enter_context(tc.tile_pool(name="psum", bufs=1, space="PSUM"))

        # ---- HAM warm-up: an uninterrupted matmul burst trips the PE
        # clock gate to 2.4 GHz while the weight DMAs stream; the dense
        # body keeps it warm.
        wdat = sb1.tile([128, 512], F16, tag="wdat")
        nc.vector.memset(wdat, 0.001)
        pwu = psum.tile([128, 512], F32, tag="ps", bufs=4, name="pwu")
        for _ in range(22):
            nc.tensor.matmul(pwu, wdat[:, 0:128], wdat, start=True, stop=True)

        # ---- persistent packs, loaded with a few large DMAs ----
        xpk = cw.tile([16, 1024], F16, tag="xpk")
        pkb = cw.tile([128, PKB_COLS], F32, tag="pkb")
        pk1 = cw.tile([128, PK1_COLS], F16, tag="pk1")
        pk2 = cw.tile([128, PK2_COLS], F16, tag="pk2")
        w1z = cw.tile([65, 256], F16, tag="w1z")
        zx = cw.tile([65, 4096], F16, tag="zx")

        nc.sync.dma_start(out=xpk, in_=ins["xpk"])
        nc.sync.dma_start(out=pkb, in_=ins["pkb"])
        for lo, hi in ((0, 512), (512, 1536), (1536, PK1_COLS)):
            nc.sync.dma_start(out=pk1[:, lo:hi], in_=ins["pk1"][:, lo:hi])
        nc.sync.dma_start(out=w1z, in_=ins["w1z"])
        nc.sync.dma_start(out=zx[64:65, :], in_=ins["xflat"])
        for lo, hi in ((0, 1366), (1366, 2732), (2732, 4096),
                       (4096, PK2_COLS)):
            nc.sync.dma_start(out=pk2[:, lo:hi], in_=ins["pk2"][:, lo:hi])

        def v1seg(name):  # pack1 2D view
            s = PK1[name]
            n = {"w1q": 512, "kw1": 512, "vw1": 512, "kw2": 256, "vw2": 256,
                 "pw": 64, "ident": 128, "blockones": 4, "ones1": 1}[name]
            return pk1[:, s:s + n]

        t = {
            "xT": xpk[:, 0:256], "w0q": xpk[:, 256:512],
            "kw0": xpk[:, 512:768], "vw0": xpk[:, 768:1024],
            "w1q": v1seg("w1q").rearrange("p (k m) -> p k m", k=2),
            "kw1": v1seg("kw1").rearrange("p (k m) -> p k m", k=2),
            "vw1": v1seg("vw1").rearrange("p (k m) -> p k m", k=2),
            "kw2": v1seg("kw2").rearrange("p (k m) -> p k m", k=2),
            "vw2": v1seg("vw2").rearrange("p (k m) -> p k m", k=2),
            "pw": v1seg("pw"), "ident": v1seg("ident"),
            "blockones": v1seg("blockones"), "ones1": v1seg("ones1"),
            "w2q": pk2[:, 0:4096].rearrange("p (k m) -> p k m", k=2),
            "dw1": pk2[:, 4096:4608].rearrange("p (k m) -> p k m", k=2),
            "dw2": pk2[:, 4608:4864].rearrange("p (k m) -> p k m", k=2),
            "w3ab": pk2[:, 4864:4868],
            "w1z": w1z,
        }
        for bname in ("qb0", "qb1", "qb2", "kb0", "kb1", "kb2", "vb0",
                      "vb1", "vb2", "b0p", "w0xpm", "db1", "db2", "hmask"):
            s = PKB[bname]
            n = {"qb2": 16, "w0xpm": 4, "hmask": 4}.get(
                bname, 1 if bname in ("kb2", "vb2", "db2") else 2)
            t[bname] = pkb[:, s:s + n]

        # ---- stage 1: query / key / value nets ----
        hq1 = sb1.tile([128, 2, 256], F16, tag="hq1")
        hq2 = sb1.tile([128, 2, 256], F16, tag="hq2")
        qT = sb1.tile([128, 16, 256], F16, tag="qT")
        kT = sb1.tile([128, 256], F16, tag="kT")
        vT = sb1.tile([128, 256], F16, tag="vT")

        for mc in range(2):
            p = psum.tile([128, 256], F32, tag="ps", bufs=4, name="p1")
            nc.tensor.matmul(p, t["w0q"][:, bass.ts(mc, 128)], t["xT"],
                             start=True, stop=True)
            nc.vector.tensor_scalar(out=hq1[:, mc], in0=p,
                                    scalar1=t["qb0"][:, mc:mc + 1],
                                    scalar2=0.0, op0=ALU.add, op1=ALU.max)
        for mc in range(2):
            p = psum.tile([128, 256], F32, tag="ps", bufs=4, name="p1")
            for kc in range(2):
                nc.tensor.matmul(p, t["w1q"][:, kc, bass.ts(mc, 128)],
                                 hq1[:, kc], start=(kc == 0), stop=(kc == 1))
            nc.vector.tensor_scalar(out=hq2[:, mc], in0=p,
                                    scalar1=t["qb1"][:, mc:mc + 1],
                                    scalar2=0.0, op0=ALU.add, op1=ALU.max)

        # k/v nets before the q output layer: w2q (1 MB) streams in last.
        for (w0, w1, w2, b0, b1, b2, outT) in (
            ("kw0", "kw1", "kw2", "kb0", "kb1", "kb2", kT),
            ("vw0", "vw1", "vw2", "vb0", "vb1", "vb2", vT),
        ):
            h1 = work.tile([128, 2, 256], F16, tag="kv1")
            h2 = work.tile([128, 2, 256], F16, tag="kv2")
            p = psum.tile([128, 1024], F32, tag="pscore", bufs=2, name="pkv")
            for mc in range(2):
                nc.tensor.matmul(p[:, bass.ts(mc, 256)],
                                 t[w0][:, bass.ts(mc, 128)], t["xT"],
                                 start=True, stop=True)
            for mc in range(2):
                nc.scalar.activation(h1[:, mc], p[:, bass.ts(mc, 256)],
                                     AF.Tanh, bias=t[b0][:, mc:mc + 1])
            p2 = psum.tile([128, 1024], F32, tag="pscore", bufs=2, name="pkv2")
            for mc in range(2):
                for kc in range(2):
                    nc.tensor.matmul(p2[:, bass.ts(mc, 256)],
                                     t[w1][:, kc, bass.ts(mc, 128)],
                                     h1[:, kc], start=(kc == 0),
                                     stop=(kc == 1))
            for mc in range(2):
                nc.scalar.activation(h2[:, mc], p2[:, bass.ts(mc, 256)],
                                     AF.Tanh, bias=t[b1][:, mc:mc + 1])
            p = psum.tile([128, 256], F32, tag="ps", bufs=4, name="p1")
            for kc in range(2):
                nc.tensor.matmul(p, t[w2][:, kc, :], h2[:, kc],
                                 start=(kc == 0), stop=(kc == 1))
            nc.vector.tensor_scalar_add(out=outT, in0=p, scalar1=t[b2][:, 0:1])

        # q output layer (MADE layer 2), fp16 weights from pack2
        for g in range(4):
            p = psum.tile([128, 1024], F32, tag="pscore", bufs=2, name="pq")
            for sub in range(4):
                mc = g * 4 + sub
                for kc in range(2):
                    nc.tensor.matmul(p[:, bass.ts(sub, 256)],
                                     t["w2q"][:, kc, bass.ts(mc, 128)],
                                     hq2[:, kc], start=(kc == 0),
                                     stop=(kc == 1))
            qb2v = bass.AP(tensor=pkb.tensor,
                           offset=pkb.offset + PKB["qb2"] + g * 4,
                           ap=[[PKB_COLS, 128], [1, 4], [0, 256]])
            nc.vector.tensor_add(qT[:, g * 4:(g + 1) * 4, :],
                                 p.rearrange("p (s n) -> p s n", n=256), qb2v)

        # Per-head masked copies of kT (other heads' rows zeroed) so the
        # scores matmuls contract over the full 128 partitions at base 0
        # (hardware rejects tile_position/base-partition offsets here).
        kTm = []
        for hh in range(4):
            km = sb1.tile([128, 256], F16, tag=f"kTm{hh}")
            nc.vector.tensor_scalar_mul(out=km, in0=kT,
                                        scalar1=t["hmask"][:, hh:hh + 1])
            kTm.append(km)

        # Diagonal scores for ALL d upfront: prodAll[.,d,.] = qT[.,d,.]*kT,
        # snn[n, 4h] per n-chunk via block-ones matmul, one exp per chunk.
        prodAll = sb1.tile([128, 16, 256], F16, tag="prodAll")
        kTb = bass.AP(tensor=kT.tensor, offset=kT.offset,
                      ap=[[256, 128], [0, 16], [1, 256]])
        nc.vector.tensor_mul(prodAll, qT, kTb)
        ed_all = []
        for ns in range(2):
            psn = psum.tile([128, 64], F32, tag="ps", bufs=4, name="psn")
            for d in range(D):
                nc.tensor.matmul(psn[:, d * 4:(d + 1) * 4],
                                 prodAll[:, d, bass.ts(ns, 128)],
                                 t["blockones"], start=True, stop=True)
            eda = sb1.tile([128, 64], F16, tag=f"ed_{ns}")
            nc.scalar.activation(eda, psn, AF.Exp)
            ed_all.append(eda)

        # v' = vT transposed, augmented with ones cols: v1_mj [128, 4*33]
        v1s = []
        for mj in range(2):
            pt = psum.tile([128, 128], F16, tag="ps", bufs=4, name="ptv")
            nc.tensor.transpose(pt, vT[:, bass.ts(mj, 128)], t["ident"])
            v1 = sb1.tile([128, 132], F16, tag=f"v1_{mj}")
            for hh in range(4):
                nc.vector.tensor_copy(out=v1[:, hh * 33:hh * 33 + 32],
                                      in_=pt[:, bass.ts(hh, 32)])
            ones_view = v1.rearrange("p (h t) -> p h t", t=33)[:, :, 32:33]
            nc.vector.tensor_copy(out=ones_view,
                                  in_=t["ones1"].to_broadcast([128, 4, 1]))
            v1s.append(v1)

        # ---- stage 2: software-pipelined attention + dimwise ----
        # Every PE consumer lags >=1 step behind its producer chain so the
        # in-order PE stream never waits on ACT/DVE latency: per step d we
        # emit scores(d), mm_o(d-1), transposes/hfeat(d-3), and staggered
        # dimwise layer stages. A dense PE stream keeps the HAM clock warm.
        yj = sb1.tile([2, D * N], F32, tag="yj")
        es_st = {}
        oA_st = {}
        oTd2_st = {}
        ag1_st = {}
        ag2_st = {}
        ag3_st = {}

        def SC2(p_):
            es = {}
            for mj in range(2):
                for half in range(2):
                    ps = psum.tile([128, 1024], F32, tag="pscore", bufs=2,
                                   name="pscore")
                    for hi in range(2):
                        hh = half * 2 + hi
                        nc.tensor.matmul(
                            ps[:, bass.ts(hi, 512)],
                            kTm[hh][:, bass.ts(mj, 128)],
                            qT[:, 2 * p_:2 * p_ + 2, :],
                            start=True, stop=True)
                    e = epool.tile([128, 1024], F16, tag="e",
                                   bufs=9, name="e")
                    nc.scalar.activation(e, ps, AF.Exp)
                    es[(mj, half)] = e
            es_st[p_] = es

        def MO(d):
            p_, di = d // 2, d % 2
            es = es_st[p_]
            poA = psum.tile([128, 264], F32, tag="ps", bufs=4, name="poA")
            for ns in range(2):
                for hh in range(4):
                    for mj in range(2):
                        e = es[(mj, hh // 2)]
                        o0 = (hh % 2) * 512 + di * 256 + ns * 128
                        nc.tensor.matmul(
                            poA[:, ns * 132 + hh * 33:ns * 132 + (hh + 1) * 33],
                            e[:, o0:o0 + 128],
                            v1s[mj][:, hh * 33:(hh + 1) * 33],
                            start=(mj == 0), stop=(mj == 1))
            if di == 1:
                del es_st[p_]
            # batched diag-correction + normalize over both n-chunks
            oAc = work.tile([128, 264], F16, tag="oAc", bufs=4, name="oAc")
            oAcv = oAc.rearrange("p (ns h t) -> p ns h t", ns=2, t=33)
            nc.vector.tensor_mul(
                oAcv[:, 0:1], v1s[0].rearrange("p (h t) -> p h t", t=33).unsqueeze(1),
                bass.AP(tensor=ed_all[0].tensor,
                        offset=ed_all[0].offset + d * 4,
                        ap=[[64, 128], [1, 1], [1, 4], [0, 33]]))
            nc.vector.tensor_mul(
                oAcv[:, 1:2], v1s[1].rearrange("p (h t) -> p h t", t=33).unsqueeze(1),
                bass.AP(tensor=ed_all[1].tensor,
                        offset=ed_all[1].offset + d * 4,
                        ap=[[64, 128], [1, 1], [1, 4], [0, 33]]))
            nc.vector.tensor_sub(oAc, oAc, poA)
            rinv = work.tile([128, 8], F32, tag="rinv", bufs=4, name="rinv")
            nc.vector.reciprocal(rinv.rearrange("p (ns h) -> p ns h", ns=2),
                                 oAcv[:, :, :, 32:33])
            oA = work.tile([128, 256], F16, tag="oA", bufs=4, name="oA")
            nc.vector.tensor_mul(
                oA.rearrange("p (ns h c) -> p ns h c", ns=2, c=32),
                oAcv[:, :, :, 0:32],
                rinv.rearrange("p (ns h) -> p ns h", ns=2).to_broadcast(
                    [128, 2, 4, 32]))
            oA_st[d] = oA

        def T(d):
            p_, di = d // 2, d % 2
            if di == 0:
                oTd2_st[p_] = work.tile([128, 512], F16, tag="oTd2", bufs=3,
                                        name="oTd2")
            oTd2 = oTd2_st[p_]
            pt = psum.tile([128, 256], F16, tag="ps", bufs=4, name="ptr")
            for ns in range(2):
                nc.tensor.transpose(pt[:, bass.ts(ns, 128)],
                                    oA_st[d][:, bass.ts(ns, 128)],
                                    t["ident"])
            nc.vector.tensor_copy(
                out=oTd2[:, di * 256:(di + 1) * 256], in_=pt)
            del oA_st[d]
            if di == 1:
                ph = psum.tile([64, 512], F32, tag="ps", bufs=4, name="ph")
                nc.tensor.matmul(ph, t["pw"], oTd2, start=True, stop=True)
                # z slice of the persistent zx tile (row 64 = x, via DMA);
                # p_b is folded into b0p on the host.
                nc.scalar.activation(zx[0:64, bass.ts(p_, 512)], ph, AF.Copy)
                del oTd2_st[p_]

        def L1(p_):
            a1 = work.tile([128, 2, 512], F16, tag="a1", bufs=2, name="a1")
            g1 = work.tile([128, 2, 512], F16, tag="g1", bufs=2, name="g1")
            for mc in range(2):
                pdm = psum.tile([128, 512], F32, tag="ps", bufs=4, name="pdm")
                nc.tensor.matmul(pdm, t["w1z"][:, bass.ts(mc, 128)],
                                 zx[:, bass.ts(p_, 512)],
                                 start=True, stop=True)
                nc.scalar.activation(a1[:, mc], pdm, AF.Tanh,
                                     bias=t["b0p"][:, mc:mc + 1])
            for mc in range(2):
                nc.gpsimd.tensor_mul(g1[:, mc], a1[:, mc], a1[:, mc])
            for mc in range(2):
                nc.gpsimd.tensor_scalar(
                    out=g1[:, mc], in0=g1[:, mc],
                    scalar1=t["w0xpm"][:, mc:mc + 1],
                    scalar2=t["w0xpm"][:, 2 + mc:3 + mc],
                    op0=ALU.mult, op1=ALU.add)
            ag1_st[p_] = (a1, g1)

        def L2(p_):
            a1, g1 = ag1_st.pop(p_)
            a2 = work.tile([128, 2, 512], F16, tag="a2", bufs=2, name="a2")
            g2 = work.tile([128, 2, 512], F16, tag="g2", bufs=2, name="g2")
            pgs = []
            for mc in range(2):
                pa = psum.tile([128, 512], F32, tag="ps", bufs=4, name="pdm")
                pg = psum.tile([128, 512], F32, tag="ps", bufs=4, name="pdg")
                for kc in range(2):
                    nc.tensor.matmul(pa, t["dw1"][:, kc, bass.ts(mc, 128)],
                                     a1[:, kc], start=(kc == 0),
                                     stop=(kc == 1))
                for kc in range(2):
                    nc.tensor.matmul(pg, t["dw1"][:, kc, bass.ts(mc, 128)],
                                     g1[:, kc], start=(kc == 0),
                                     stop=(kc == 1))
                nc.scalar.activation(a2[:, mc], pa, AF.Tanh,
                                     bias=t["db1"][:, mc:mc + 1])
                pgs.append(pg)
            for mc in range(2):
                nc.gpsimd.tensor_mul(g2[:, mc], a2[:, mc], a2[:, mc])
            for mc in range(2):
                nc.vector.scalar_tensor_tensor(
                    out=g2[:, mc], in0=g2[:, mc], scalar=1.0, in1=pgs[mc],
                    op0=ALU.subtract, op1=ALU.mult)
            ag2_st[p_] = (a2, g2)

        def L3(p_):
            a2, g2 = ag2_st.pop(p_)
            a3 = work.tile([128, 512], F16, tag="a3", bufs=2, name="a3")
            g3 = work.tile([128, 512], F16, tag="g3", bufs=2, name="g3")
            pa = psum.tile([128, 512], F32, tag="ps", bufs=4, name="pdm")
            pg = psum.tile([128, 512], F32, tag="ps", bufs=4, name="pdg")
            for kc in range(2):
                nc.tensor.matmul(pa, t["dw2"][:, kc, :], a2[:, kc],
                                 start=(kc == 0), stop=(kc == 1))
            for kc in range(2):
                nc.tensor.matmul(pg, t["dw2"][:, kc, :], g2[:, kc],
                                 start=(kc == 0), stop=(kc == 1))
            nc.scalar.activation(a3, pa, AF.Tanh, bias=t["db2"][:, 0:1])
            nc.gpsimd.tensor_mul(g3, a3, a3)
            nc.vector.scalar_tensor_tensor(
                out=g3, in0=g3, scalar=1.0, in1=pg,
                op0=ALU.subtract, op1=ALU.mult)
            ag3_st[p_] = (a3, g3)

        def L4(p_):
            a3, g3 = ag3_st.pop(p_)
            py = psum.tile([2, 512], F32, tag="ps", bufs=4, name="py")
            nc.tensor.matmul(py, t["w3ab"][:, 0:2], a3, start=True,
                             stop=False)
            nc.tensor.matmul(py, t["w3ab"][:, 2:4], g3, start=False,
                             stop=True)
            nc.vector.tensor_copy(out=yj[:, bass.ts(p_, 512)], in_=py)

        # schedule: SC(d); MO(d-1); T(d-3); L1(p)@d=2p+4, L2@2p+5,
        # L3@2p+6, L4@2p+7
        def dw_stages(d):
            for stage, off in ((L1, 4), (L2, 5), (L3, 6), (L4, 7)):
                if d >= off and (d - off) % 2 == 0:
                    p_ = (d - off) // 2
                    if p_ < D // 2:
                        stage(p_)

        for d in range(0, D + 9):
            if d % 2 == 0 and d // 2 < D // 2:
                SC2(d // 2)
            if 0 <= d - 1 < D:
                MO(d - 1)
            if 0 <= d - 2 < D:
                T(d - 2)
            dw_stages(d)
        nc.sync.dma_start(out=out, in_=yj)


def _build():
    nc = bacc.Bacc("TRN2", target_bir_lowering=False, debug=False)
    shapes = {
        "xpk": ([16, 1024], F16), "pkb": ([128, PKB_COLS], F32),
        "pk1": ([128, PK1_COLS], F16), "pk2": ([128, PK2_COLS], F16),
        "w1z": ([65, 256], F16), "xflat": ([1, D * N], F16),
    }
    ins = {n: nc.dram_tensor(n, s, dt, kind="ExternalInput").ap()
           for n, (s, dt) in shapes.items()}
    out = nc.dram_tensor("out", [2, D * N], F32, kind="ExternalOutput").ap()
    with tile.TileContext(nc) as tc:
        _emit(nc, tc, ins, out)
    nc.finalize()
    return nc


def _col2(v):
    # [256] -> [128, 2] with column mc = chunk mc
    return np.ascontiguousarray(v.reshape(2, 128).T)


def _prep_inputs(t, x, q_w0, q_b0, k_w0, k_b0, v_w0, v_b0, q_w1, q_b1, k_w1,
                 k_b1, v_w1, v_b1, q_w2, q_b2, k_w2, k_b2, v_w2, v_b2, p_w,
                 p_b, d_w0, d_b0, d_w1, d_b1, d_w2, d_b2, d_w3, d_b3, q_m0,
                 q_m1, q_m2):
    f = np.float32
    f16 = np.float16
    scale = f(1.0 / np.sqrt(dh))

    def kchunk(w):  # [256, M] -> [128, 2*M] (row chunk-major)
        return np.ascontiguousarray(
            w.reshape(2, 128, -1).transpose(1, 0, 2).reshape(128, -1))

    W0q = (q_w0 * q_m0).astype(f)
    W1q = (q_w1 * q_m1).astype(f)
    W2q = (q_w2 * q_m2).astype(f)
    # b0p folds the t-channel AND the p_b projection bias contribution
    b0p = (d_b0 + t[0] * d_w0[0] + p_b @ d_w0[2:66]).astype(f)
    w0x = d_w0[1].astype(f)
    W1z = np.concatenate([d_w0[2:], d_w0[1:2]], axis=0).astype(f)
    w3ab = np.zeros((H, 4), f)
    w3ab[:, 0] = d_w3[:, 0]
    w3ab[:, 3] = d_w3[:, 0]
    w0xpm = np.concatenate([_col2(-w0x), _col2(w0x)], axis=1)

    pk1 = np.concatenate([
        kchunk(W1q), kchunk(k_w1.astype(f)), kchunk(v_w1.astype(f)),
        kchunk((k_w2 * scale).astype(f)), kchunk(v_w2.astype(f)),
        p_w.astype(f), np.eye(128, dtype=f),
        np.repeat(np.eye(4, dtype=f), 32, axis=0),
        np.ones((128, 1), f),
    ], axis=1).astype(f16)
    pk2 = np.concatenate([
        kchunk(W2q), kchunk(d_w1.astype(f)), kchunk(d_w2.astype(f)), w3ab,
    ], axis=1).astype(f16)
    pkb = np.concatenate([
        _col2(q_b0.astype(f)), _col2(q_b1.astype(f)),
        np.ascontiguousarray(q_b2.astype(f).reshape(16, 128).T),
        _col2(k_b0.astype(f)), _col2(k_b1.astype(f)),
        (k_b2 * scale).astype(f).reshape(128, 1),
        _col2(v_b0.astype(f)), _col2(v_b1.astype(f)),
        v_b2.astype(f).reshape(128, 1),
        _col2(b0p), w0xpm,
        _col2(d_b1.astype(f)), d_b2.astype(f).reshape(128, 1),
        np.repeat(np.eye(4, dtype=f), 32, axis=0),
    ], axis=1)
    w1z16 = W1z.astype(f16)

    shared = {
        "pk1": np.ascontiguousarray(pk1),
        "pk2": np.ascontiguousarray(pk2),
        "pkb": np.ascontiguousarray(pkb),
        "w1z": np.ascontiguousarray(w1z16),
    }
    in_maps = []
    for b in range(B):
        m = dict(shared)
        xt = np.ascontiguousarray(x[b].T.astype(f)).astype(f16)
        m["xpk"] = np.ascontiguousarray(np.concatenate(
            [xt, W0q.astype(f16), k_w0.astype(f).astype(f16),
             v_w0.astype(f).astype(f16)], axis=1))
        m["xflat"] = np.ascontiguousarray(xt.reshape(1, -1))
        in_maps.append(m)
    return in_maps, float(d_b3[0])


def kernel(**inputs):
    from concourse.bass_utils import run_bass_kernel_spmd

    inputs = {k: np.asarray(v) for k, v in inputs.items()}
    with _lock:
        if "nc" not in _cache:
            _cache["nc"] = _build()
        nc = _cache["nc"]
    in_maps, b3 = _prep_inputs(**inputs)
    trace = False
    if os.environ.get("KBENCH_TRACE"):
        try:
            import antenv.axon_hooks  # noqa: F401
            trace = True
        except ImportError:
            trace = False
    res = run_bass_kernel_spmd(nc, in_maps, list(range(B)), trace=trace)
    if trace:
        _cache["last_results"] = res
    y = np.zeros((B, N, D), np.float32)
    jac = np.zeros((B, N, D), np.float32)
    for b in range(B):
        o = res.results[b]["out"].reshape(2, D, N)
        y[b] = o[0].T + np.float32(b3)
        jac[b] = o[1].T
    return y, jac


# revision 16
# speedup vs baseline: 1.3093x; 1.0467x over previous
"""Trainium2 Bass kernel for nn_DiffeqExactTraceAttention.

Strategy: data-parallel over batch B=8 across the 8 NeuronCores (one batch
element per core, attention over N=256 fully local, weights replicated).

Per-core computation (all activations stored transposed, [feat, token]):
  query MADE-MLP -> qT [2048, 256]; k/v tanh-MLPs -> kT, vT [128, 256]
  per dim d (16): scoresT[m,n] per head via PE, exp on ACT (no max needed:
  |scores| < 1), diagonal correction via separately-computed diag scores,
  o + softmax denominator from one matmul against [v' | 1], per-partition
  normalize, PE transpose, projection + dimwise 4-layer MLP forward and
  JVP (diagonal Jacobian).

All matmul/DVE traffic is fp16 (PSUM accumulation stays fp32); weights are
packed into two fp16 SBUF blobs + one fp32 bias blob loaded with a handful
of large DMAs so the weight load stays off the critical path.

Outputs y, jac [B, N, D] (d_b3 added host-side).
"""

import os
import sys
import threading

import numpy as np

sys.path.insert(0, "/opt/trn_rl_repo")

import concourse.bass as bass  # noqa: E402
import concourse.mybir as mybir  # noqa: E402
import concourse.tile as tile  # noqa: E402
from concourse import bacc  # noqa: E402

F32 = mybir.dt.float32
F16 = mybir.dt.float16
AF = mybir.ActivationFunctionType
ALU = mybir.AluOpType

B, N, D = 8, 256, 16
HID, H, DH, NH = 256, 128, 64, 4
dh = H // NH  # 32

_lock = threading.Lock()
_cache = {}

# fp16 pack1 segment offsets (cols)
PK1 = {"w1q": 0, "kw1": 512, "vw1": 1024, "kw2": 1536, "vw2": 1792,
       "pw": 2048, "ident": 2112, "blockones": 2240, "ones1": 2244}
PK1_COLS = 2245
# fp16 pack2 segment offsets
PK2 = {"w2q": 0, "dw1": 4096, "dw2": 4608, "w3ab": 4864}
PK2_COLS = 4868
# fp32 bias pack column offsets
PKB = {"qb0": 0, "qb1": 2, "qb2": 4, "kb0": 20, "kb1": 22, "kb2": 24,
       "vb0": 25, "vb1": 27, "vb2": 29, "b0p": 30, "w0xpm": 32,
       "db1": 36, "db2": 38, "hmask": 39}
PKB_COLS = 43


def _emit(nc, tc, ins, out):
    """Emit the per-core kernel. `ins` maps name -> dram AP."""
    from contextlib import ExitStack

    with ExitStack() as ctx:
        cw = ctx.enter_context(tc.tile_pool(name="cw", bufs=1))
        sb1 = ctx.enter_context(tc.tile_pool(name="sb1", bufs=1))
        work = ctx.enter_context(tc.tile_pool(name="work", bufs=3))
        epool = ctx.enter_context(tc.tile_pool(name="epool", bufs=3))
        psum = ctx.enter_context(tc.tile_pool(name="psum", bufs=1, space="PSUM"))

        # ---- HAM warm-up: an uninterrupted matmul burst trips the PE
        # clock gate to 2.4 GHz while the weight DMAs stream; the dense
        # body keeps it warm.
        wdat = sb1.tile([128, 512], F16, tag="wdat")
        nc.vector.memset(wdat, 0.001)
        pwu = psum.tile([128, 512], F32, tag="ps", bufs=4, name="pwu")
        for _ in range(16):
            nc.tensor.matmul(pwu, wdat[:, 0:128], wdat, start=True, stop=True)

        # ---- persistent packs, loaded with a few large DMAs ----
        xpk = cw.tile([16, 1024], F16, tag="xpk")
        pkb = cw.tile([128, PKB_COLS], F32, tag="pkb")
        pk1 = cw.tile([128, PK1_COLS], F16, tag="pk1")
        pk2 = cw.tile([128, PK2_COLS], F16, tag="pk2")
        w1z = cw.tile([66, 256], F16, tag="w1z")
        zx = cw.tile([66, 4096], F16, tag="zx")

        nc.sync.dma_start(out=xpk, in_=ins["xpk"])
        nc.sync.dma_start(out=pkb, in_=ins["pkb"])
        for lo, hi in ((0, 512), (512, 1536), (1536, PK1_COLS)):
            nc.sync.dma_start(out=pk1[:, lo:hi], in_=ins["pk1"][:, lo:hi])
        nc.sync.dma_start(out=w1z, in_=ins["w1z"])
        nc.sync.dma_start(out=zx[64:66, :], in_=ins["xflat"])
        for lo, hi in ((0, 1366), (1366, 2732), (2732, 4096),
                       (4096, PK2_COLS)):
            nc.sync.dma_start(out=pk2[:, lo:hi], in_=ins["pk2"][:, lo:hi])

        def v1seg(name):  # pack1 2D view
            s = PK1[name]
            n = {"w1q": 512, "kw1": 512, "vw1": 512, "kw2": 256, "vw2": 256,
                 "pw": 64, "ident": 128, "blockones": 4, "ones1": 1}[name]
            return pk1[:, s:s + n]

        t = {
            "xT": xpk[:, 0:256], "w0q": xpk[:, 256:512],
            "kw0": xpk[:, 512:768], "vw0": xpk[:, 768:1024],
            "w1q": v1seg("w1q").rearrange("p (k m) -> p k m", k=2),
            "kw1": v1seg("kw1").rearrange("p (k m) -> p k m", k=2),
            "vw1": v1seg("vw1").rearrange("p (k m) -> p k m", k=2),
            "kw2": v1seg("kw2").rearrange("p (k m) -> p k m", k=2),
            "vw2": v1seg("vw2").rearrange("p (k m) -> p k m", k=2),
            "pw": v1seg("pw"), "ident": v1seg("ident"),
            "blockones": v1seg("blockones"), "ones1": v1seg("ones1"),
            "w2q": pk2[:, 0:4096].rearrange("p (k m) -> p k m", k=2),
            "dw1": pk2[:, 4096:4608].rearrange("p (k m) -> p k m", k=2),
            "dw2": pk2[:, 4608:4864].rearrange("p (k m) -> p k m", k=2),
            "w3ab": pk2[:, 4864:4868],
            "w1z": w1z,
        }
        for bname in ("qb0", "qb1", "qb2", "kb0", "kb1", "kb2", "vb0",
                      "vb1", "vb2", "b0p", "w0xpm", "db1", "db2", "hmask"):
            s = PKB[bname]
            n = {"qb2": 16, "w0xpm": 4, "hmask": 4}.get(
                bname, 1 if bname in ("kb2", "vb2", "db2") else 2)
            t[bname] = pkb[:, s:s + n]

        # ---- stage 1: query / key / value nets ----
        hq1 = sb1.tile([128, 2, 256], F16, tag="hq1")
        hq2 = sb1.tile([128, 2, 256], F16, tag="hq2")
        qT = sb1.tile([128, 16, 256], F16, tag="qT")
        kT = sb1.tile([128, 256], F16, tag="kT")
        vT = sb1.tile([128, 256], F16, tag="vT")

        for mc in range(2):
            p = psum.tile([128, 256], F32, tag="ps", bufs=4, name="p1")
            nc.tensor.matmul(p, t["w0q"][:, bass.ts(mc, 128)], t["xT"],
                             start=True, stop=True)
            nc.vector.tensor_scalar(out=hq1[:, mc], in0=p,
                                    scalar1=t["qb0"][:, mc:mc + 1],
                                    scalar2=0.0, op0=ALU.add, op1=ALU.max)
        for mc in range(2):
            p = psum.tile([128, 256], F32, tag="ps", bufs=4, name="p1")
            for kc in range(2):
                nc.tensor.matmul(p, t["w1q"][:, kc, bass.ts(mc, 128)],
                                 hq1[:, kc], start=(kc == 0), stop=(kc == 1))
            nc.vector.tensor_scalar(out=hq2[:, mc], in0=p,
                                    scalar1=t["qb1"][:, mc:mc + 1],
                                    scalar2=0.0, op0=ALU.add, op1=ALU.max)

        # k/v nets before the q output layer: w2q (1 MB) streams in last.
        for (w0, w1, w2, b0, b1, b2, outT) in (
            ("kw0", "kw1", "kw2", "kb0", "kb1", "kb2", kT),
            ("vw0", "vw1", "vw2", "vb0", "vb1", "vb2", vT),
        ):
            h1 = work.tile([128, 2, 256], F16, tag="kv1")
            h2 = work.tile([128, 2, 256], F16, tag="kv2")
            p = psum.tile([128, 1024], F32, tag="pscore", bufs=2, name="pkv")
            for mc in range(2):
                nc.tensor.matmul(p[:, bass.ts(mc, 256)],
                                 t[w0][:, bass.ts(mc, 128)], t["xT"],
                                 start=True, stop=True)
            for mc in range(2):
                nc.scalar.activation(h1[:, mc], p[:, bass.ts(mc, 256)],
                                     AF.Tanh, bias=t[b0][:, mc:mc + 1])
            p2 = psum.tile([128, 1024], F32, tag="pscore", bufs=2, name="pkv2")
            for mc in range(2):
                for kc in range(2):
                    nc.tensor.matmul(p2[:, bass.ts(mc, 256)],
                                     t[w1][:, kc, bass.ts(mc, 128)],
                                     h1[:, kc], start=(kc == 0),
                                     stop=(kc == 1))
            for mc in range(2):
                nc.scalar.activation(h2[:, mc], p2[:, bass.ts(mc, 256)],
                                     AF.Tanh, bias=t[b1][:, mc:mc + 1])
            p = psum.tile([128, 256], F32, tag="ps", bufs=4, name="p1")
            for kc in range(2):
                nc.tensor.matmul(p, t[w2][:, kc, :], h2[:, kc],
                                 start=(kc == 0), stop=(kc == 1))
            nc.vector.tensor_scalar_add(out=outT, in0=p, scalar1=t[b2][:, 0:1])

        # q output layer (MADE layer 2), fp16 weights from pack2
        for g in range(4):
            p = psum.tile([128, 1024], F32, tag="pscore", bufs=2, name="pq")
            for sub in range(4):
                mc = g * 4 + sub
                for kc in range(2):
                    nc.tensor.matmul(p[:, bass.ts(sub, 256)],
                                     t["w2q"][:, kc, bass.ts(mc, 128)],
                                     hq2[:, kc], start=(kc == 0),
                                     stop=(kc == 1))
            qb2v = bass.AP(tensor=pkb.tensor,
                           offset=pkb.offset + PKB["qb2"] + g * 4,
                           ap=[[PKB_COLS, 128], [1, 4], [0, 256]])
            nc.vector.tensor_add(qT[:, g * 4:(g + 1) * 4, :],
                                 p.rearrange("p (s n) -> p s n", n=256), qb2v)

        # Diagonal scores for ALL d upfront: prodAll[.,d,.] = qT[.,d,.]*kT,
        # snn[n, 4h] per n-chunk via block-ones matmul, one exp per chunk.
        prodAll = sb1.tile([128, 16, 256], F16, tag="prodAll")
        kTb = bass.AP(tensor=kT.tensor, offset=kT.offset,
                      ap=[[256, 128], [0, 16], [1, 256]])
        nc.vector.tensor_mul(prodAll, qT, kTb)
        ed_all = []
        for ns in range(2):
            psn = psum.tile([128, 64], F32, tag="ps", bufs=4, name="psn")
            for d in range(D):
                nc.tensor.matmul(psn[:, d * 4:(d + 1) * 4],
                                 prodAll[:, d, bass.ts(ns, 128)],
                                 t["blockones"], start=True, stop=True)
            eda = sb1.tile([128, 64], F16, tag=f"ed_{ns}")
            nc.scalar.activation(eda, psn, AF.Exp)
            ed_all.append(eda)

        # v' = vT transposed, augmented with ones cols: v1_mj [128, 4*33]
        v1s = []
        for mj in range(2):
            pt = psum.tile([128, 128], F16, tag="ps", bufs=4, name="ptv")
            nc.tensor.transpose(pt, vT[:, bass.ts(mj, 128)], t["ident"])
            v1 = sb1.tile([128, 132], F16, tag=f"v1_{mj}")
            for hh in range(4):
                nc.vector.tensor_copy(out=v1[:, hh * 33:hh * 33 + 32],
                                      in_=pt[:, bass.ts(hh, 32)])
            ones_view = v1.rearrange("p (h t) -> p h t", t=33)[:, :, 32:33]
            nc.vector.tensor_copy(out=ones_view,
                                  in_=t["ones1"].to_broadcast([128, 4, 1]))
            v1s.append(v1)

        # ---- stage 2: software-pipelined attention + dimwise ----
        # Every PE consumer lags >=1 step behind its producer chain so the
        # in-order PE stream never waits on ACT/DVE latency: per step d we
        # emit scores(d), mm_o(d-1), transposes/hfeat(d-3), and staggered
        # dimwise layer stages. A dense PE stream keeps the HAM clock warm.
        yj = sb1.tile([2, D * N], F32, tag="yj")
        es_st = {}
        oA_st = {}
        oTd2_st = {}
        ag1_st = {}
        ag2_st = {}
        ag3_st = {}

        def SC2(p_):
            es = {}
            for mj in range(2):
                for half in range(2):
                    ps = psum.tile([128, 1024], F32, tag="pscore", bufs=2,
                                   name="pscore")
                    for hi in range(2):
                        hh = half * 2 + hi
                        # per-head 32-row contraction placed at PE array
                        # row 32*hh via tile_position (smaller LDWEIGHTS,
                        # no masked kT copies needed)
                        nc.tensor.matmul(
                            ps[:, bass.ts(hi, 512)],
                            kT[32 * hh:32 * (hh + 1), bass.ts(mj, 128)],
                            qT[32 * hh:32 * (hh + 1), 2 * p_:2 * p_ + 2, :],
                            start=True, stop=True,
                            tile_position=(32 * hh, 0))
                    e = epool.tile([128, 1024], F16, tag="e",
                                   bufs=9, name="e")
                    nc.scalar.activation(e, ps, AF.Exp)
                    es[(mj, half)] = e
            es_st[p_] = es

        def MO(d):
            p_, di = d // 2, d % 2
            es = es_st[p_]
            poA = psum.tile([128, 264], F32, tag="ps", bufs=4, name="poA")
            for ns in range(2):
                for hh in range(4):
                    for mj in range(2):
                        e = es[(mj, hh // 2)]
                        o0 = (hh % 2) * 512 + di * 256 + ns * 128
                        nc.tensor.matmul(
                            poA[:, ns * 132 + hh * 33:ns * 132 + (hh + 1) * 33],
                            e[:, o0:o0 + 128],
                            v1s[mj][:, hh * 33:(hh + 1) * 33],
                            start=(mj == 0), stop=(mj == 1))
            if di == 1:
                del es_st[p_]
            # batched diag-correction + normalize over both n-chunks
            oAc = work.tile([128, 264], F16, tag="oAc", bufs=4, name="oAc")
            oAcv = oAc.rearrange("p (ns h t) -> p ns h t", ns=2, t=33)
            nc.vector.tensor_mul(
                oAcv[:, 0:1], v1s[0].rearrange("p (h t) -> p h t", t=33).unsqueeze(1),
                bass.AP(tensor=ed_all[0].tensor,
                        offset=ed_all[0].offset + d * 4,
                        ap=[[64, 128], [1, 1], [1, 4], [0, 33]]))
            nc.vector.tensor_mul(
                oAcv[:, 1:2], v1s[1].rearrange("p (h t) -> p h t", t=33).unsqueeze(1),
                bass.AP(tensor=ed_all[1].tensor,
                        offset=ed_all[1].offset + d * 4,
                        ap=[[64, 128], [1, 1], [1, 4], [0, 33]]))
            nc.vector.tensor_sub(oAc, oAc, poA)
            rinv = work.tile([128, 8], F32, tag="rinv", bufs=4, name="rinv")
            nc.vector.reciprocal(rinv.rearrange("p (ns h) -> p ns h", ns=2),
                                 oAcv[:, :, :, 32:33])
            oA = work.tile([128, 256], F16, tag="oA", bufs=4, name="oA")
            nc.vector.tensor_mul(
                oA.rearrange("p (ns h c) -> p ns h c", ns=2, c=32),
                oAcv[:, :, :, 0:32],
                rinv.rearrange("p (ns h) -> p ns h", ns=2).to_broadcast(
                    [128, 2, 4, 32]))
            oA_st[d] = oA

        def T(d):
            p_, di = d // 2, d % 2
            if di == 0:
                oTd2_st[p_] = work.tile([128, 512], F16, tag="oTd2", bufs=3,
                                        name="oTd2")
            oTd2 = oTd2_st[p_]
            pt = psum.tile([128, 256], F16, tag="ps", bufs=4, name="ptr")
            for ns in range(2):
                nc.tensor.transpose(pt[:, bass.ts(ns, 128)],
                                    oA_st[d][:, bass.ts(ns, 128)],
                                    t["ident"])
            nc.vector.tensor_copy(
                out=oTd2[:, di * 256:(di + 1) * 256], in_=pt)
            del oA_st[d]
            if di == 1:
                ph = psum.tile([64, 512], F32, tag="ps", bufs=4, name="ph")
                nc.tensor.matmul(ph, t["pw"], oTd2, start=True, stop=True)
                # z slice of the persistent zx tile (rows 64/65 = x/ones,
                # via DMA); p_b is folded into the w1z bias row on the host.
                nc.vector.tensor_copy(out=zx[0:64, bass.ts(p_, 512)], in_=ph)
                del oTd2_st[p_]

        def L1(p_):
            # bias comes from the w1z ones-row (row 65) -> single wide tanh
            a1 = work.tile([128, 2, 512], F16, tag="a1", bufs=2, name="a1")
            g1 = work.tile([128, 2, 512], F16, tag="g1", bufs=2, name="g1")
            pdm = psum.tile([128, 1024], F32, tag="pscore", bufs=2,
                            name="pdm1")
            for mc in range(2):
                nc.tensor.matmul(pdm[:, bass.ts(mc, 512)],
                                 t["w1z"][:, bass.ts(mc, 128)],
                                 zx[:, bass.ts(p_, 512)],
                                 start=True, stop=True)
            nc.scalar.activation(a1.rearrange("p a b -> p (a b)"), pdm,
                                 AF.Tanh)
            nc.vector.tensor_mul(g1, a1, a1)
            for mc in range(2):
                nc.gpsimd.tensor_scalar(
                    out=g1[:, mc], in0=g1[:, mc],
                    scalar1=t["w0xpm"][:, mc:mc + 1],
                    scalar2=t["w0xpm"][:, 2 + mc:3 + mc],
                    op0=ALU.mult, op1=ALU.add)
            ag1_st[p_] = (a1, g1)

        def L2(p_):
            a1, g1 = ag1_st.pop(p_)
            a2 = work.tile([128, 2, 512], F16, tag="a2", bufs=2, name="a2")
            g2 = work.tile([128, 2, 512], F16, tag="g2", bufs=2, name="g2")
            pgs = []
            for mc in range(2):
                pa = psum.tile([128, 512], F32, tag="ps", bufs=4, name="pdm")
                pg = psum.tile([128, 512], F32, tag="ps", bufs=4, name="pdg")
                for kc in range(2):
                    nc.tensor.matmul(pa, t["dw1"][:, kc, bass.ts(mc, 128)],
                                     a1[:, kc], start=(kc == 0),
                                     stop=(kc == 1))
                for kc in range(2):
                    nc.tensor.matmul(pg, t["dw1"][:, kc, bass.ts(mc, 128)],
                                     g1[:, kc], start=(kc == 0),
                                     stop=(kc == 1))
                nc.scalar.activation(a2[:, mc], pa, AF.Tanh,
                                     bias=t["db1"][:, mc:mc + 1])
                pgs.append(pg)
            nc.vector.tensor_mul(g2, a2, a2)
            for mc in range(2):
                nc.vector.scalar_tensor_tensor(
                    out=g2[:, mc], in0=g2[:, mc], scalar=1.0, in1=pgs[mc],
                    op0=ALU.subtract, op1=ALU.mult)
            ag2_st[p_] = (a2, g2)

        def L3(p_):
            a2, g2 = ag2_st.pop(p_)
            a3 = work.tile([128, 512], F16, tag="a3", bufs=2, name="a3")
            g3 = work.tile([128, 512], F16, tag="g3", bufs=2, name="g3")
            pa = psum.tile([128, 512], F32, tag="ps", bufs=4, name="pdm")
            pg = psum.tile([128, 512], F32, tag="ps", bufs=4, name="pdg")
            for kc in range(2):
                nc.tensor.matmul(pa, t["dw2"][:, kc, :], a2[:, kc],
                                 start=(kc == 0), stop=(kc == 1))
            for kc in range(2):
                nc.tensor.matmul(pg, t["dw2"][:, kc, :], g2[:, kc],
                                 start=(kc == 0), stop=(kc == 1))
            nc.scalar.activation(a3, pa, AF.Tanh, bias=t["db2"][:, 0:1])
            nc.vector.tensor_mul(g3, a3, a3)
            nc.vector.scalar_tensor_tensor(
                out=g3, in0=g3, scalar=1.0, in1=pg,
                op0=ALU.subtract, op1=ALU.mult)
            ag3_st[p_] = (a3, g3)

        def L4(p_):
            a3, g3 = ag3_st.pop(p_)
            py = psum.tile([2, 512], F32, tag="ps", bufs=4, name="py")
            nc.tensor.matmul(py, t["w3ab"][:, 0:2], a3, start=True,
                             stop=False)
            nc.tensor.matmul(py, t["w3ab"][:, 2:4], g3, start=False,
                             stop=True)
            nc.vector.tensor_copy(out=yj[:, bass.ts(p_, 512)], in_=py)

        # schedule: SC(d); MO(d-1); T(d-2); L1(p)@d=2p+4, L2@2p+6,
        # L3@2p+7, L4@2p+8 (L2 lags L1 by a full d-pair so the PE never
        # waits on the a1 -> g1 elementwise chain)
        def dw_stages(d):
            for stage, off in ((L1, 4), (L2, 6), (L3, 7), (L4, 8)):
                if d >= off and (d - off) % 2 == 0:
                    p_ = (d - off) // 2
                    if p_ < D // 2:
                        stage(p_)

        for d in range(0, D + 9):
            if d % 2 == 0 and d // 2 < D // 2:
                SC2(d // 2)
            if 0 <= d - 1 < D:
                MO(d - 1)
            if 0 <= d - 2 < D:
                T(d - 2)
            dw_stages(d)
        nc.sync.dma_start(out=out, in_=yj)


def _build():
    nc = bacc.Bacc("TRN2", target_bir_lowering=False, debug=False)
    shapes = {
        "xpk": ([16, 1024], F16), "pkb": ([128, PKB_COLS], F32),
        "pk1": ([128, PK1_COLS], F16), "pk2": ([128, PK2_COLS], F16),
        "w1z": ([66, 256], F16), "xflat": ([2, D * N], F16),
    }
    ins = {n: nc.dram_tensor(n, s, dt, kind="ExternalInput").ap()
           for n, (s, dt) in shapes.items()}
    out = nc.dram_tensor("out", [2, D * N], F32, kind="ExternalOutput").ap()
    with tile.TileContext(nc) as tc:
        _emit(nc, tc, ins, out)
    nc.finalize()
    return nc


def _col2(v):
    # [256] -> [128, 2] with column mc = chunk mc
    return np.ascontiguousarray(v.reshape(2, 128).T)


def _prep_inputs(t, x, q_w0, q_b0, k_w0, k_b0, v_w0, v_b0, q_w1, q_b1, k_w1,
                 k_b1, v_w1, v_b1, q_w2, q_b2, k_w2, k_b2, v_w2, v_b2, p_w,
                 p_b, d_w0, d_b0, d_w1, d_b1, d_w2, d_b2, d_w3, d_b3, q_m0,
                 q_m1, q_m2):
    f = np.float32
    f16 = np.float16
    scale = f(1.0 / np.sqrt(dh))

    def kchunk(w):  # [256, M] -> [128, 2*M] (row chunk-major)
        return np.ascontiguousarray(
            w.reshape(2, 128, -1).transpose(1, 0, 2).reshape(128, -1))

    W0q = (q_w0 * q_m0).astype(f)
    W1q = (q_w1 * q_m1).astype(f)
    W2q = (q_w2 * q_m2).astype(f)
    # b0p folds the t-channel AND the p_b projection bias contribution
    b0p = (d_b0 + t[0] * d_w0[0] + p_b @ d_w0[2:66]).astype(f)
    w0x = d_w0[1].astype(f)
    W1z = np.concatenate([d_w0[2:], d_w0[1:2], b0p[None, :]],
                         axis=0).astype(f)
    w3ab = np.zeros((H, 4), f)
    w3ab[:, 0] = d_w3[:, 0]
    w3ab[:, 3] = d_w3[:, 0]
    w0xpm = np.concatenate([_col2(-w0x), _col2(w0x)], axis=1)

    pk1 = np.concatenate([
        kchunk(W1q), kchunk(k_w1.astype(f)), kchunk(v_w1.astype(f)),
        kchunk((k_w2 * scale).astype(f)), kchunk(v_w2.astype(f)),
        p_w.astype(f), np.eye(128, dtype=f),
        np.repeat(np.eye(4, dtype=f), 32, axis=0),
        np.ones((128, 1), f),
    ], axis=1).astype(f16)
    pk2 = np.concatenate([
        kchunk(W2q), kchunk(d_w1.astype(f)), kchunk(d_w2.astype(f)), w3ab,
    ], axis=1).astype(f16)
    pkb = np.concatenate([
        _col2(q_b0.astype(f)), _col2(q_b1.astype(f)),
        np.ascontiguousarray(q_b2.astype(f).reshape(16, 128).T),
        _col2(k_b0.astype(f)), _col2(k_b1.astype(f)),
        (k_b2 * scale).astype(f).reshape(128, 1),
        _col2(v_b0.astype(f)), _col2(v_b1.astype(f)),
        v_b2.astype(f).reshape(128, 1),
        _col2(b0p), w0xpm,
        _col2(d_b1.astype(f)), d_b2.astype(f).reshape(128, 1),
        np.repeat(np.eye(4, dtype=f), 32, axis=0),
    ], axis=1)
    w1z16 = W1z.astype(f16)

    shared = {
        "pk1": np.ascontiguousarray(pk1),
        "pk2": np.ascontiguousarray(pk2),
        "pkb": np.ascontiguousarray(pkb),
        "w1z": np.ascontiguousarray(w1z16),
    }
    in_maps = []
    for b in range(B):
        m = dict(shared)
        xt = np.ascontiguousarray(x[b].T.astype(f)).astype(f16)
        m["xpk"] = np.ascontiguousarray(np.concatenate(
            [xt, W0q.astype(f16), k_w0.astype(f).astype(f16),
             v_w0.astype(f).astype(f16)], axis=1))
        m["xflat"] = np.ascontiguousarray(np.concatenate(
            [xt.reshape(1, -1), np.ones((1, D * N), f16)], axis=0))
        in_maps.append(m)
    return in_maps, float(d_b3[0])


def kernel(**inputs):
    from concourse.bass_utils import run_bass_kernel_spmd

    inputs = {k: np.asarray(v) for k, v in inputs.items()}
    with _lock:
        if "nc" not in _cache:
            _cache["nc"] = _build()
        nc = _cache["nc"]
    in_maps, b3 = _prep_inputs(**inputs)
    trace = False
    if os.environ.get("KBENCH_TRACE"):
        try:
            import antenv.axon_hooks  # noqa: F401
            trace = True
        except ImportError:
            trace = False
    res = run_bass_kernel_spmd(nc, in_maps, list(range(B)), trace=trace)
    if trace:
        _cache["last_results"] = res
    y = np.zeros((B, N, D), np.float32)
    jac = np.zeros((B, N, D), np.float32)
    for b in range(B):
        o = res.results[b]["out"].reshape(2, D, N)
        y[b] = o[0].T + np.float32(b3)
        jac[b] = o[1].T
    return y, jac
